# revision 7
# baseline (speedup 1.0000x reference)
"""Bahdanau (MLP) attention kernel for Trainium2, data-parallel over batch.

reference math (per batch b):
    q_proj = query @ Wq + bq                     [Lq, H]
    k_proj = memory @ Wm                         [Lm, H]
    attn[q, m] = sum_h v[h] * tanh(q_proj[q, h] + k_proj[m, h])
    attn = where(mask[m], -1e24, attn)
    weights = softmax(attn, axis=-1)             [Lq, Lm]
    weighted_memory = weights @ memory           [Lq, Ms]
    returns (weighted_memory, weights)

Shapes are hardcoded: B=8, Lq=128, Lm=512, Q=M=512, H=256, fp32.
One batch per NeuronCore (8 cores).

Device layout (per core):
  - k_projT [h, m] and q_projT [h, q] with h on partitions (2 chunks of 128).
  - main loop over q: z = k_projT + q_projT[:, q] (DVE tensor_scalar, fp32 2x),
    tanh via one big ACT instruction per group of GQ q's (fp16 out),
    attn row accumulation on PE: lhsT = [128, 32] fp16 "masked v" (v in column
    q%32, zeros elsewhere) so each matmul writes one 32-partition strip of the
    attn PSUM bank; zeros accumulate exactly.
  - epilogue: +mask, rowwise softmax, PE transpose of weights, fp32 matmul
    weights @ memory.
"""

import functools
import os

import numpy as np

B, LQ, LM = 8, 128, 512
Q_SIZE, M_SIZE, H_SIZE = 512, 512, 256
MASKED_VALUE = -1e24
P = 128
HC = H_SIZE // P  # 2 h-chunks
DC = Q_SIZE // P  # 4 d-chunks
MC = LM // P      # 4 m-chunks
GQ = 4            # q's per tanh batch
NG = LQ // GQ
QSTRIP = 32       # PE col-tiling strip


def _build_nc():
    import concourse.mybir as mybir
    import concourse.tile as tile
    from concourse import bacc
    from concourse.masks import make_identity

    f32 = mybir.dt.float32
    f16 = mybir.dt.float16
    AF = mybir.ActivationFunctionType
    AX = mybir.AxisListType

    nc = bacc.Bacc("TRN2", name="mlp_attn")

    q_d = nc.dram_tensor("query", [LQ, Q_SIZE], f32, kind="ExternalInput")
    m_d = nc.dram_tensor("memory", [LM, M_SIZE], f32, kind="ExternalInput")
    mask_d = nc.dram_tensor("maskval", [LQ, LM], f32, kind="ExternalInput")
    wq_d = nc.dram_tensor("Wq", [Q_SIZE, H_SIZE], f32, kind="ExternalInput")
    wm_d = nc.dram_tensor("Wm", [M_SIZE, H_SIZE], f32, kind="ExternalInput")
    bq_d = nc.dram_tensor("bqc", [P, HC], f32, kind="ExternalInput")
    vmask_d = nc.dram_tensor("vmask", [P, HC, QSTRIP, QSTRIP], f16, kind="ExternalInput")
    wmo_d = nc.dram_tensor("wm_out", [LQ, M_SIZE], f32, kind="ExternalOutput")
    wo_d = nc.dram_tensor("w_out", [LQ, LM], f32, kind="ExternalOutput")

    with tile.TileContext(nc) as tc:
        with (
            tc.tile_pool(name="const", bufs=1) as cpool,
            tc.tile_pool(name="io", bufs=1) as iopool,
            tc.tile_pool(name="work", bufs=1) as wpool,
            tc.tile_pool(name="z", bufs=2) as zpool,
            tc.tile_pool(name="th", bufs=2) as thpool,
            tc.tile_pool(name="ps", bufs=2, space="PSUM") as pspool,
            tc.tile_pool(name="tp", bufs=2, space="PSUM") as tppool,
            tc.tile_pool(name="attnps", bufs=1, space="PSUM") as apool,
            tc.tile_pool(name="outps", bufs=1, space="PSUM") as opool,
        ):
            # ---------------- constants + input DMA ----------------
            ident = cpool.tile([P, P], f32)
            make_identity(nc, ident[:])

            bq_sb = cpool.tile([P, HC], f32)
            nc.sync.dma_start(bq_sb[:], bq_d[:])

            # masked-v stationaries: vmask[hc][r][:, c] = v_chunk_hc if c == r else 0
            # (prepared host-side; one [128, 32] fp16 lhsT per (hc, q%32))
            vmask = cpool.tile([P, HC, QSTRIP, QSTRIP], f16)
            nc.sync.dma_start(vmask[:], vmask_d[:])

            wq_sb = iopool.tile([P, DC, H_SIZE], f32)
            nc.sync.dma_start(wq_sb[:], wq_d.rearrange("(dc p) h -> p dc h", p=P))
            wm_sb = iopool.tile([P, DC, H_SIZE], f32)
            nc.sync.dma_start(wm_sb[:], wm_d.rearrange("(dc p) h -> p dc h", p=P))
            qry_sb = iopool.tile([P, Q_SIZE], f32)
            nc.sync.dma_start(qry_sb[:], q_d[:])
            mem_sb = iopool.tile([P, MC, M_SIZE], f32)
            for mc in range(MC):
                nc.sync.dma_start(mem_sb[:, mc, :], m_d[mc * P : (mc + 1) * P, :])
            maskbc = iopool.tile([P, LM], f32)
            nc.sync.dma_start(maskbc[:], mask_d[:])

            # ---------------- transposes: queryT [d, q], memoryT [d, m] ----------------
            qryT = wpool.tile([P, DC, LQ], f32)
            for dc in range(DC):
                tp = tppool.tile([P, P], f32)
                nc.tensor.transpose(tp[:], qry_sb[:, dc * P : (dc + 1) * P], ident[:])
                nc.vector.tensor_copy(qryT[:, dc, :], tp[:])
            memT = wpool.tile([P, DC, LM], f32)
            for mc in range(MC):
                for dc in range(DC):
                    tp = tppool.tile([P, P], f32)
                    nc.tensor.transpose(
                        tp[:], mem_sb[:, mc, dc * P : (dc + 1) * P], ident[:]
                    )
                    nc.vector.tensor_copy(memT[:, dc, mc * P : (mc + 1) * P], tp[:])

            # ---------------- projections: kpT [h, m], qpT [h, q] (+bq) ----------------
            kpT = wpool.tile([P, HC, LM], f32)
            for hc in range(HC):
                pt = pspool.tile([P, LM], f32, tag="proj_psum")
                for dc in range(DC):
                    nc.tensor.matmul(
                        pt[:],
                        wm_sb[:, dc, hc * P : (hc + 1) * P],
                        memT[:, dc, :],
                        start=(dc == 0),
                        stop=(dc == DC - 1),
                    )
                nc.vector.tensor_copy(kpT[:, hc, :], pt[:])
            qpT = wpool.tile([P, HC, LQ], f32)
            for hc in range(HC):
                pt = pspool.tile([P, LM], f32, tag="proj_psum")
                for dc in range(DC):
                    nc.tensor.matmul(
                        pt[:, :LQ],
                        wq_sb[:, dc, hc * P : (hc + 1) * P],
                        qryT[:, dc, :],
                        start=(dc == 0),
                        stop=(dc == DC - 1),
                    )
                nc.scalar.activation(
                    qpT[:, hc, :], pt[:, :LQ], AF.Identity, bias=bq_sb[:, hc : hc + 1]
                )

            # ---------------- main loop: attn[q, m] in one PSUM bank ----------------
            attn_ps = apool.tile([P, LM], f32)
            for g in range(NG):
                z = zpool.tile([P, GQ * HC, LM], f32)
                for ql in range(GQ):
                    q = g * GQ + ql
                    for hc in range(HC):
                        nc.vector.tensor_scalar_add(
                            z[:, ql * HC + hc, :],
                            kpT[:, hc, :],
                            qpT[:, hc, q : q + 1],
                        )
                th = thpool.tile([P, GQ * HC, LM], f16)
                nc.scalar.activation(th[:], z[:], AF.Tanh)
                for ql in range(GQ):
                    q = g * GQ + ql
                    s, r = divmod(q, QSTRIP)
                    for hc in range(HC):
                        first = (q % QSTRIP == 0) and hc == 0
                        last = (q % QSTRIP == QSTRIP - 1) and hc == HC - 1
                        nc.tensor.matmul(
                            attn_ps[s * QSTRIP : (s + 1) * QSTRIP, :],
                            vmask[:, hc, r, :],
                            th[:, ql * HC + hc, :],
                            start=first,
                            stop=last,
                            tile_position=(0, s * QSTRIP),
                        )

            # ---------------- softmax ----------------
            attn_sb = wpool.tile([P, LM], f32)
            nc.vector.tensor_add(attn_sb[:], attn_ps[:], maskbc[:])
            mx = wpool.tile([P, 1], f32)
            nc.vector.reduce_max(mx[:], attn_sb[:], axis=AX.X)
            negmx = wpool.tile([P, 1], f32)
            nc.vector.tensor_scalar_mul(negmx[:], mx[:], -1.0)
            esb = wpool.tile([P, LM], f32)
            nc.scalar.activation(esb[:], attn_sb[:], AF.Exp, bias=negmx[:])
            sm = wpool.tile([P, 1], f32)
            nc.vector.reduce_sum(sm[:], esb[:], axis=AX.X)
            rs = wpool.tile([P, 1], f32)
            nc.vector.reciprocal(rs[:], sm[:])
            w_sb = wpool.tile([P, LM], f32)
            nc.vector.tensor_scalar_mul(w_sb[:], esb[:], rs[:])
            nc.sync.dma_start(wo_d[:], w_sb[:])

            # ---------------- weighted_memory = weights @ memory ----------------
            wT = wpool.tile([P, MC, LQ], f32)
            for mc in range(MC):
                tp = tppool.tile([P, P], f32)
                nc.tensor.transpose(tp[:], w_sb[:, mc * P : (mc + 1) * P], ident[:])
                nc.vector.tensor_copy(wT[:, mc, :], tp[:])
            out_ps = opool.tile([P, M_SIZE], f32)
            for mc in range(MC):
                nc.tensor.matmul(
                    out_ps[:],
                    wT[:, mc, :],
                    mem_sb[:, mc, :],
                    start=(mc == 0),
                    stop=(mc == MC - 1),
                )
            out_sb = wpool.tile([P, M_SIZE], f32)
            nc.vector.tensor_copy(out_sb[:], out_ps[:])
            nc.sync.dma_start(wmo_d[:], out_sb[:])

    nc.compile()
    return nc


@functools.lru_cache(maxsize=1)
def _get_nc():
    return _build_nc()


def _prep_in_maps(query, memory, mask, Wq, bq, Wm, v):
    query = np.ascontiguousarray(np.asarray(query, dtype=np.float32))
    memory = np.ascontiguousarray(np.asarray(memory, dtype=np.float32))
    mask = np.asarray(mask)
    Wq = np.ascontiguousarray(np.asarray(Wq, dtype=np.float32))
    Wm = np.ascontiguousarray(np.asarray(Wm, dtype=np.float32))
    bq = np.asarray(bq, dtype=np.float32)
    v = np.asarray(v, dtype=np.float32)

    maskval = np.where(mask, np.float32(MASKED_VALUE), np.float32(0.0)).astype(
        np.float32
    )  # [B, LM]
    maskbc = np.ascontiguousarray(
        np.broadcast_to(maskval[:, None, :], (B, LQ, LM))
    ).astype(np.float32)
    bqc = np.ascontiguousarray(bq.reshape(HC, P).T)  # [P, HC]
    vc = np.ascontiguousarray(v.reshape(HC, P).T)  # [P, HC]
    vmask = np.zeros((P, HC, QSTRIP, QSTRIP), dtype=np.float16)
    idx = np.arange(QSTRIP)
    vmask[:, :, idx, idx] = vc[:, :, None]

    in_maps = []
    for b in range(B):
        in_maps.append(
            {
                "query": query[b],
                "memory": memory[b],
                "maskval": maskbc[b],
                "Wq": Wq,
                "Wm": Wm,
                "bqc": bqc,
                "vmask": vmask,
            }
        )
    return in_maps


def _run(inputs, trace=False):
    """Run on 8 NeuronCores; returns ((weighted_memory, weights), exec_time_ns)."""
    from concourse.bass_utils import run_bass_kernel_spmd

    nc = _get_nc()
    in_maps = _prep_in_maps(**inputs)
    res = run_bass_kernel_spmd(
        nc, in_maps, core_ids=list(range(B)), trace=trace
    )
    wm = np.stack([r["wm_out"] for r in res.results]).astype(np.float32)
    w = np.stack([r["w_out"] for r in res.results]).astype(np.float32)
    return (wm, w), res.exec_time_ns


def kernel(query, memory, mask, Wq, bq, Wm, v):
    (wm, w), _ = _run(
        dict(query=query, memory=memory, mask=mask, Wq=Wq, bq=bq, Wm=Wm, v=v),
        trace=bool(int(os.environ.get("KERNEL_TRACE", "0"))),
    )
    return wm, w


if __name__ == "__main__":
    nc = _get_nc()
    print("built ok:", nc.name)


# revision 15
# speedup vs baseline: 1.0634x; 1.0634x over previous
"""Bahdanau (MLP) attention kernel for Trainium2, data-parallel over batch.

reference math (per batch b):
    q_proj = query @ Wq + bq                     [Lq, H]
    k_proj = memory @ Wm                         [Lm, H]
    attn[q, m] = sum_h v[h] * tanh(q_proj[q, h] + k_proj[m, h])
    attn = where(mask[m], -1e24, attn)
    weights = softmax(attn, axis=-1)             [Lq, Lm]
    weighted_memory = weights @ memory           [Lq, Ms]
    returns (weighted_memory, weights)

Shapes are hardcoded: B=8, Lq=128, Lm=512, Q=M=512, H=256, fp32.
One batch per NeuronCore (8 cores).

Device layout (per core):
  - k_projT [h, m] and q_projT [h, q] with h on partitions (2 chunks of 128).
  - main loop over q: z = k_projT + q_projT[:, q] (DVE tensor_scalar, fp32 2x),
    tanh via one big ACT instruction per group of GQ q's (fp16 out),
    attn row accumulation on PE: lhsT = [128, 32] fp16 "masked v" (v in column
    q%32, zeros elsewhere) so each matmul writes one 32-partition strip of the
    attn PSUM bank; zeros accumulate exactly.
  - epilogue: +mask, rowwise softmax, PE transpose of weights, fp32 matmul
    weights @ memory.
"""

import functools
import os

import numpy as np

B, LQ, LM = 8, 128, 512
Q_SIZE, M_SIZE, H_SIZE = 512, 512, 256
MASKED_VALUE = -1e24
P = 128
HC = H_SIZE // P  # 2 h-chunks
DC = Q_SIZE // P  # 4 d-chunks
MC = LM // P      # 4 m-chunks
GQ = 4            # q's per tanh batch
NG = LQ // GQ
QSTRIP = 32       # PE col-tiling strip


def _build_nc():
    import concourse.mybir as mybir
    import concourse.tile as tile
    from concourse import bacc
    from concourse.masks import make_identity

    f32 = mybir.dt.float32
    f32r = mybir.dt.float32r
    f16 = mybir.dt.float16
    AF = mybir.ActivationFunctionType
    AX = mybir.AxisListType

    nc = bacc.Bacc("TRN2", name="mlp_attn")

    q_d = nc.dram_tensor("query", [LQ, Q_SIZE], f32r, kind="ExternalInput")
    m_d = nc.dram_tensor("memory", [LM, M_SIZE], f32r, kind="ExternalInput")
    mask_d = nc.dram_tensor("maskval", [LQ, LM], f32, kind="ExternalInput")
    wq_d = nc.dram_tensor("Wq", [Q_SIZE, H_SIZE], f32r, kind="ExternalInput")
    wm_d = nc.dram_tensor("Wm", [M_SIZE, H_SIZE], f32r, kind="ExternalInput")
    bq_d = nc.dram_tensor("bqc", [P, HC], f32, kind="ExternalInput")
    vmask_d = nc.dram_tensor("vmask", [P, HC, QSTRIP, QSTRIP], f16, kind="ExternalInput")
    wmo_d = nc.dram_tensor("wm_out", [LQ, M_SIZE], f32, kind="ExternalOutput")
    wo_d = nc.dram_tensor("w_out", [LQ, LM], f32, kind="ExternalOutput")

    with tile.TileContext(nc) as tc:
        with (
            tc.tile_pool(name="const", bufs=1) as cpool,
            tc.tile_pool(name="io", bufs=1) as iopool,
            tc.tile_pool(name="work", bufs=1) as wpool,
            tc.tile_pool(name="z", bufs=2) as zpool,
            tc.tile_pool(name="th", bufs=2) as thpool,
            tc.tile_pool(name="ps", bufs=2, space="PSUM") as pspool,
            tc.tile_pool(name="tp", bufs=2, space="PSUM") as tppool,
            tc.tile_pool(name="attnps", bufs=1, space="PSUM") as apool,
            tc.tile_pool(name="outps", bufs=1, space="PSUM") as opool,
        ):
            # ---------------- constants + input DMA ----------------
            ident = cpool.tile([P, P], f32)
            make_identity(nc, ident[:])
            ident_r = cpool.tile([P, P], f32r)
            nc.vector.tensor_copy(ident_r[:], ident[:])

            bq_sb = cpool.tile([P, HC], f32)
            nc.sync.dma_start(bq_sb[:], bq_d[:])

            # masked-v stationaries: vmask[hc][r][:, c] = v_chunk_hc if c == r else 0
            # (prepared host-side; one [128, 32] fp16 lhsT per (hc, q%32))
            vmask = cpool.tile([P, HC, QSTRIP, QSTRIP], f16)
            nc.sync.dma_start(vmask[:], vmask_d[:])

            wq_sb = iopool.tile([P, DC, H_SIZE], f32r)
            nc.sync.dma_start(wq_sb[:], wq_d.rearrange("(dc p) h -> p dc h", p=P))
            wm_sb = iopool.tile([P, DC, H_SIZE], f32r)
            nc.sync.dma_start(wm_sb[:], wm_d.rearrange("(dc p) h -> p dc h", p=P))
            qry_sb = iopool.tile([P, Q_SIZE], f32r)
            nc.sync.dma_start(qry_sb[:], q_d[:])
            mem_sb = iopool.tile([P, MC, M_SIZE], f32r)
            for mc in range(MC):
                nc.sync.dma_start(mem_sb[:, mc, :], m_d[mc * P : (mc + 1) * P, :])
            maskbc = iopool.tile([P, LM], f32)
            nc.sync.dma_start(maskbc[:], mask_d[:])

            # ---------------- transposes: queryT [d, q]; qpT [h, q] (+bq) ----------------
            qryT = wpool.tile([P, DC, LQ], f32r)
            for dc in range(DC):
                tp = tppool.tile([P, P], f32r, tag="tp")
                nc.tensor.transpose(tp[:], qry_sb[:, dc * P : (dc + 1) * P], ident_r[:])
                nc.vector.tensor_copy(qryT[:, dc, :], tp[:])
            qpT = wpool.tile([P, HC, LQ], f32)
            for hc in range(HC):
                pt = pspool.tile([P, LM], f32, tag="proj_psum")
                for dc in range(DC):
                    nc.tensor.matmul(
                        pt[:, :LQ],
                        wq_sb[:, dc, hc * P : (hc + 1) * P],
                        qryT[:, dc, :],
                        start=(dc == 0),
                        stop=(dc == DC - 1),
                    )
                nc.scalar.activation(
                    qpT[:, hc, :], pt[:, :LQ], AF.Identity, bias=bq_sb[:, hc : hc + 1]
                )

            # ---------------- memoryT [d, m] + projections kpT [h, m] (f16) ----------
            memT = wpool.tile([P, DC, LM], f32r)
            for dc in range(DC):
                for mc in range(MC):
                    tp = tppool.tile([P, P], f32r, tag="tp")
                    nc.tensor.transpose(
                        tp[:], mem_sb[:, mc, dc * P : (dc + 1) * P], ident_r[:]
                    )
                    nc.vector.tensor_copy(memT[:, dc, mc * P : (mc + 1) * P], tp[:])
            kpT = wpool.tile([P, HC, LM], f16)
            for hc in range(HC):
                pt = pspool.tile([P, LM], f32, tag="proj_psum")
                for dc in range(DC):
                    nc.tensor.matmul(
                        pt[:],
                        wm_sb[:, dc, hc * P : (hc + 1) * P],
                        memT[:, dc, :],
                        start=(dc == 0),
                        stop=(dc == DC - 1),
                    )
                nc.vector.tensor_copy(kpT[:, hc, :], pt[:])

            # ---------------- main loop: attn[q, m] in one PSUM bank ----------------
            attn_ps = apool.tile([P, LM], f32)
            for g in range(NG):
                z = zpool.tile([P, GQ * HC, LM], f16)
                for ql in range(GQ):
                    q = g * GQ + ql
                    for hc in range(HC):
                        nc.vector.tensor_scalar_add(
                            z[:, ql * HC + hc, :],
                            kpT[:, hc, :],
                            qpT[:, hc, q : q + 1],
                        )
                th = thpool.tile([P, GQ * HC, LM], f16)
                nc.scalar.activation(th[:], z[:], AF.Tanh)
                for ql in range(GQ):
                    q = g * GQ + ql
                    s, r = divmod(q, QSTRIP)
                    for hc in range(HC):
                        first = (q % QSTRIP == 0) and hc == 0
                        last = (q % QSTRIP == QSTRIP - 1) and hc == HC - 1
                        nc.tensor.matmul(
                            attn_ps[s * QSTRIP : (s + 1) * QSTRIP, :],
                            vmask[:, hc, r, :],
                            th[:, ql * HC + hc, :],
                            start=first,
                            stop=last,
                            tile_position=(0, s * QSTRIP),
                        )

            # ---------------- softmax ----------------
            attn_sb = wpool.tile([P, LM], f32)
            nc.vector.tensor_add(attn_sb[:], attn_ps[:], maskbc[:])
            mx = wpool.tile([P, 1], f32)
            nc.vector.reduce_max(mx[:], attn_sb[:], axis=AX.X)
            negmx = wpool.tile([P, 1], f32)
            nc.vector.tensor_scalar_mul(negmx[:], mx[:], -1.0)
            esb = wpool.tile([P, LM], f32)
            nc.scalar.activation(esb[:], attn_sb[:], AF.Exp, bias=negmx[:])
            sm = wpool.tile([P, 1], f32)
            nc.vector.reduce_sum(sm[:], esb[:], axis=AX.X)
            rs = wpool.tile([P, 1], f32)
            nc.vector.reciprocal(rs[:], sm[:])
            w_sb = wpool.tile([P, LM], f32)
            nc.vector.tensor_scalar_mul(w_sb[:], esb[:], rs[:])
            nc.sync.dma_start(wo_d[:], w_sb[:])

            # ---------------- weighted_memory = weights @ memory ----------------
            wT = wpool.tile([P, MC, LQ], f32r)
            for mc in range(MC):
                tp = tppool.tile([P, P], f32, tag="tpw")
                nc.tensor.transpose(tp[:], w_sb[:, mc * P : (mc + 1) * P], ident[:])
                nc.vector.tensor_copy(wT[:, mc, :], tp[:])
            out_ps = opool.tile([P, M_SIZE], f32)
            for mc in range(MC):
                nc.tensor.matmul(
                    out_ps[:],
                    wT[:, mc, :],
                    mem_sb[:, mc, :],
                    start=(mc == 0),
                    stop=(mc == MC - 1),
                )
            out_sb = wpool.tile([P, M_SIZE], f32)
            nc.vector.tensor_copy(out_sb[:], out_ps[:])
            nc.sync.dma_start(wmo_d[:], out_sb[:])

    nc.compile()
    return nc


@functools.lru_cache(maxsize=1)
def _get_nc():
    return _build_nc()


def _prep_in_maps(query, memory, mask, Wq, bq, Wm, v):
    query = np.ascontiguousarray(np.asarray(query, dtype=np.float32))
    memory = np.ascontiguousarray(np.asarray(memory, dtype=np.float32))
    mask = np.asarray(mask)
    Wq = np.ascontiguousarray(np.asarray(Wq, dtype=np.float32))
    Wm = np.ascontiguousarray(np.asarray(Wm, dtype=np.float32))
    bq = np.asarray(bq, dtype=np.float32)
    v = np.asarray(v, dtype=np.float32)

    maskval = np.where(mask, np.float32(MASKED_VALUE), np.float32(0.0)).astype(
        np.float32
    )  # [B, LM]
    maskbc = np.ascontiguousarray(
        np.broadcast_to(maskval[:, None, :], (B, LQ, LM))
    ).astype(np.float32)
    bqc = np.ascontiguousarray(bq.reshape(HC, P).T)  # [P, HC]
    vc = np.ascontiguousarray(v.reshape(HC, P).T)  # [P, HC]
    vmask = np.zeros((P, HC, QSTRIP, QSTRIP), dtype=np.float16)
    idx = np.arange(QSTRIP)
    vmask[:, :, idx, idx] = vc[:, :, None]

    in_maps = []
    for b in range(B):
        in_maps.append(
            {
                "query": query[b],
                "memory": memory[b],
                "maskval": maskbc[b],
                "Wq": Wq,
                "Wm": Wm,
                "bqc": bqc,
                "vmask": vmask,
            }
        )
    return in_maps


def _run(inputs, trace=False):
    """Run on 8 NeuronCores; returns ((weighted_memory, weights), exec_time_ns)."""
    from concourse.bass_utils import run_bass_kernel_spmd

    nc = _get_nc()
    in_maps = _prep_in_maps(**inputs)
    res = run_bass_kernel_spmd(
        nc, in_maps, core_ids=list(range(B)), trace=trace
    )
    wm = np.stack([r["wm_out"] for r in res.results]).astype(np.float32)
    w = np.stack([r["w_out"] for r in res.results]).astype(np.float32)
    return (wm, w), res.exec_time_ns


def kernel(query, memory, mask, Wq, bq, Wm, v):
    (wm, w), _ = _run(
        dict(query=query, memory=memory, mask=mask, Wq=Wq, bq=bq, Wm=Wm, v=v),
        trace=bool(int(os.environ.get("KERNEL_TRACE", "0"))),
    )
    return wm, w


if __name__ == "__main__":
    nc = _get_nc()
    print("built ok:", nc.name)


# revision 21
# speedup vs baseline: 1.4421x; 1.3561x over previous
"""Bahdanau (MLP) attention kernel for Trainium2, data-parallel over batch.

reference math (per batch b):
    q_proj = query @ Wq + bq                     [Lq, H]
    k_proj = memory @ Wm                         [Lm, H]
    attn[q, m] = sum_h v[h] * tanh(q_proj[q, h] + k_proj[m, h])
    attn = where(mask[m], -1e24, attn)
    weights = softmax(attn, axis=-1)             [Lq, Lm]
    weighted_memory = weights @ memory           [Lq, Ms]
    returns (weighted_memory, weights)

Shapes hardcoded: B=8, Lq=128, Lm=512, Q=M=512, H=256, fp32. One batch per
NeuronCore (8 cores, SPMD).

Masked memory positions receive softmax weight exactly 0 (exp(-1e24) == 0 in
fp32), so their tanh columns never affect either output. The host gathers the
unmasked memory rows (a mask-derived layout transform), the device computes
attention over MU = ceil(max_unmasked/128)*128 compacted columns, and the host
scatters the compact weights back to [Lq, Lm] (masked entries = 0 exactly, as
in the reference). weighted_memory comes out of the device already full-width.

Device pipeline (per core):
  - k_projT [h, mu] and q_projT [h, q] with h on partitions (2 chunks of 128),
    via PE transposes + fp32r matmuls (query/memory/Wq/Wm declared float32r).
  - main loop over groups of GQ q's: DVE pre-adds z = k_projT + q_projT[:, q]
    (fp16 out, 4x mode), one big ACT tanh per group (fp16), PE accumulates
    attn[q, mu] = sum_h v_h * tanh into one PSUM bank using [128, 32] fp16
    "masked v" stationaries (v in column q%32) so each matmul writes one
    32-partition strip; zero columns accumulate exactly.
  - epilogue: +pad-mask, softmax without max-subtraction (|attn| < sum|v| < 16
    so exp cannot overflow; -1e24 still underflows to 0), PE transpose of
    weights, fp32r matmul weights @ memory_compact -> full [Lq, Ms].
"""

import functools
import os

import numpy as np

B, LQ, LM = 8, 128, 512
Q_SIZE, M_SIZE, H_SIZE = 512, 512, 256
MASKED_VALUE = -1e24
P = 128
HC = H_SIZE // P  # 2 h-chunks
DC = Q_SIZE // P  # 4 d-chunks
GQ = 4            # q's per tanh batch
NG = LQ // GQ
QSTRIP = 32       # PE col-tiling strip


def _build_nc(MU):
    import concourse.mybir as mybir
    import concourse.tile as tile
    from concourse import bacc
    from concourse.masks import make_identity

    f32 = mybir.dt.float32
    f32r = mybir.dt.float32r
    f16 = mybir.dt.float16
    AF = mybir.ActivationFunctionType
    AX = mybir.AxisListType

    MUC = MU // P  # compacted m-chunks

    nc = bacc.Bacc("TRN2", name="mlp_attn")

    q_d = nc.dram_tensor("query", [LQ, Q_SIZE], f32r, kind="ExternalInput")
    m_d = nc.dram_tensor("memory", [MU, M_SIZE], f32r, kind="ExternalInput")
    mask_d = nc.dram_tensor("maskval", [LQ, MU], f32, kind="ExternalInput")
    wq_d = nc.dram_tensor("Wq", [Q_SIZE, H_SIZE], f32r, kind="ExternalInput")
    wm_d = nc.dram_tensor("Wm", [M_SIZE, H_SIZE], f32r, kind="ExternalInput")
    bq_d = nc.dram_tensor("bqc", [P, HC], f32, kind="ExternalInput")
    vmask_d = nc.dram_tensor("vmask", [P, HC, QSTRIP, QSTRIP], f16, kind="ExternalInput")
    wmo_d = nc.dram_tensor("wm_out", [LQ, M_SIZE], f32, kind="ExternalOutput")
    wo_d = nc.dram_tensor("w_out", [LQ, MU], f32, kind="ExternalOutput")

    with tile.TileContext(nc) as tc:
        with (
            tc.tile_pool(name="const", bufs=1) as cpool,
            tc.tile_pool(name="io", bufs=1) as iopool,
            tc.tile_pool(name="work", bufs=1) as wpool,
            tc.tile_pool(name="z", bufs=3) as zpool,
            tc.tile_pool(name="th", bufs=3) as thpool,
            tc.tile_pool(name="ps", bufs=2, space="PSUM") as pspool,
            tc.tile_pool(name="tp", bufs=3, space="PSUM") as tppool,
            tc.tile_pool(name="attnps", bufs=1, space="PSUM") as apool,
            tc.tile_pool(name="outps", bufs=1, space="PSUM") as opool,
        ):
            # ---------------- constants ----------------
            ident = cpool.tile([P, P], f32)
            make_identity(nc, ident[:])
            ident_r = cpool.tile([P, P], f32r)
            nc.vector.tensor_copy(ident_r[:], ident[:])

            # preload the exp_and_others ACT table set (tanh + exp) at t=0
            warm = cpool.tile([P, 1], f32)
            nc.vector.memset(warm[:], 0.0)
            nc.scalar.activation(warm[:], warm[:], AF.Tanh)

            # DMA order matches PE consumption order: query -> Wq -> memory -> Wm
            qry_sb = iopool.tile([P, Q_SIZE], f32r)
            nc.sync.dma_start(qry_sb[:], q_d[:])
            wq_sb = iopool.tile([P, DC, H_SIZE], f32r)
            nc.sync.dma_start(wq_sb[:], wq_d.rearrange("(dc p) h -> p dc h", p=P))
            mem_sb = iopool.tile([P, MUC, M_SIZE], f32r)
            for mc in range(MUC):
                nc.sync.dma_start(mem_sb[:, mc, :], m_d[mc * P : (mc + 1) * P, :])
            wm_sb = iopool.tile([P, DC, H_SIZE], f32r)
            nc.sync.dma_start(wm_sb[:], wm_d.rearrange("(dc p) h -> p dc h", p=P))
            bq_sb = cpool.tile([P, HC], f32)
            nc.sync.dma_start(bq_sb[:], bq_d[:])

            # masked-v stationaries: vmask[hc][r][:, c] = v_chunk_hc if c == r else 0
            vmask = cpool.tile([P, HC, QSTRIP, QSTRIP], f16)
            nc.sync.dma_start(vmask[:], vmask_d[:])
            maskbc = iopool.tile([P, MU], f32)
            nc.sync.dma_start(maskbc[:], mask_d[:])

            # ---------------- queryT [d, q]; qpT [h, q] (+bq) ----------------
            qryT = wpool.tile([P, DC, LQ], f32r)
            for dc in range(DC):
                tp = tppool.tile([P, P], f32r, tag="tp")
                nc.tensor.transpose(tp[:], qry_sb[:, dc * P : (dc + 1) * P], ident_r[:])
                nc.scalar.copy(qryT[:, dc, :], tp[:])
            qpT = wpool.tile([P, HC, LQ], f32)
            for hc in range(HC):
                pt = pspool.tile([P, M_SIZE], f32, tag="proj_psum")
                for dc in range(DC):
                    nc.tensor.matmul(
                        pt[:, :LQ],
                        wq_sb[:, dc, hc * P : (hc + 1) * P],
                        qryT[:, dc, :],
                        start=(dc == 0),
                        stop=(dc == DC - 1),
                    )
                nc.scalar.activation(
                    qpT[:, hc, :], pt[:, :LQ], AF.Identity, bias=bq_sb[:, hc : hc + 1]
                )

            # ---------------- memoryT [d, mu] + kpT [h, mu] (f16) ----------
            memT = wpool.tile([P, DC, MU], f32r)
            for dc in range(DC):
                for mc in range(MUC):
                    tp = tppool.tile([P, P], f32r, tag="tp")
                    nc.tensor.transpose(
                        tp[:], mem_sb[:, mc, dc * P : (dc + 1) * P], ident_r[:]
                    )
                    nc.scalar.copy(memT[:, dc, mc * P : (mc + 1) * P], tp[:])
            kpT = wpool.tile([P, HC, MU], f16)
            for hc in range(HC):
                pt = pspool.tile([P, M_SIZE], f32, tag="proj_psum")
                for dc in range(DC):
                    nc.tensor.matmul(
                        pt[:, :MU],
                        wm_sb[:, dc, hc * P : (hc + 1) * P],
                        memT[:, dc, :],
                        start=(dc == 0),
                        stop=(dc == DC - 1),
                    )
                nc.scalar.copy(kpT[:, hc, :], pt[:, :MU])

            # ---------------- main loop: attn[q, mu] in one PSUM bank ----------------
            attn_ps = apool.tile([P, MU], f32)
            for g in range(NG):
                z = zpool.tile([P, GQ * HC, MU], f16)
                for ql in range(GQ):
                    q = g * GQ + ql
                    for hc in range(HC):
                        nc.vector.tensor_scalar_add(
                            z[:, ql * HC + hc, :],
                            kpT[:, hc, :],
                            qpT[:, hc, q : q + 1],
                        )
                th = thpool.tile([P, GQ * HC, MU], f16)
                nc.scalar.activation(th[:], z[:], AF.Tanh)
                for ql in range(GQ):
                    q = g * GQ + ql
                    s, r = divmod(q, QSTRIP)
                    for hc in range(HC):
                        first = (q % QSTRIP == 0) and hc == 0
                        last = (q % QSTRIP == QSTRIP - 1) and hc == HC - 1
                        nc.tensor.matmul(
                            attn_ps[s * QSTRIP : (s + 1) * QSTRIP, :],
                            vmask[:, hc, r, :],
                            th[:, ql * HC + hc, :],
                            start=first,
                            stop=last,
                            tile_position=(0, s * QSTRIP),
                        )

            # ---------------- softmax (no max-subtraction; |attn| < 16) ----------
            attn_sb = wpool.tile([P, MU], f32)
            nc.vector.tensor_add(attn_sb[:], attn_ps[:], maskbc[:])
            esb = wpool.tile([P, MU], f32)
            nc.scalar.activation(esb[:], attn_sb[:], AF.Exp)
            sm = wpool.tile([P, 1], f32)
            nc.vector.reduce_sum(sm[:], esb[:], axis=AX.X)
            rs = wpool.tile([P, 1], f32)
            nc.vector.reciprocal(rs[:], sm[:])
            w_sb = wpool.tile([P, MU], f32)
            nc.vector.tensor_scalar_mul(w_sb[:], esb[:], rs[:])
            nc.sync.dma_start(wo_d[:], w_sb[:])

            # ---------------- weighted_memory = weights @ memory_compact --------
            wT = wpool.tile([P, MUC, LQ], f32r)
            for mc in range(MUC):
                tp = tppool.tile([P, P], f32, tag="tp")
                nc.tensor.transpose(tp[:], w_sb[:, mc * P : (mc + 1) * P], ident[:])
                nc.vector.tensor_copy(wT[:, mc, :], tp[:])
            out_ps = opool.tile([P, M_SIZE], f32)
            for mc in range(MUC):
                nc.tensor.matmul(
                    out_ps[:],
                    wT[:, mc, :],
                    mem_sb[:, mc, :],
                    start=(mc == 0),
                    stop=(mc == MUC - 1),
                )
            out_sb = wpool.tile([P, M_SIZE], f32)
            nc.vector.tensor_copy(out_sb[:], out_ps[:])
            nc.sync.dma_start(wmo_d[:], out_sb[:])

    nc.compile()
    return nc


@functools.lru_cache(maxsize=2)
def _get_nc(MU=LM):
    return _build_nc(MU)


def _choose_mu(mask):
    """Smallest multiple of 128 covering every batch's unmasked count."""
    mu_max = int((~mask).sum(axis=-1).max())
    mu = max(P, -(-mu_max // P) * P)
    return min(mu, LM)


def _prep_in_maps(query, memory, mask, Wq, bq, Wm, v, MU):
    query = np.ascontiguousarray(np.asarray(query, dtype=np.float32))
    memory = np.ascontiguousarray(np.asarray(memory, dtype=np.float32))
    mask = np.asarray(mask).astype(bool)
    Wq = np.ascontiguousarray(np.asarray(Wq, dtype=np.float32))
    Wm = np.ascontiguousarray(np.asarray(Wm, dtype=np.float32))
    bq = np.asarray(bq, dtype=np.float32)
    v = np.asarray(v, dtype=np.float32)

    bqc = np.ascontiguousarray(bq.reshape(HC, P).T)  # [P, HC]
    vc = np.ascontiguousarray(v.reshape(HC, P).T)  # [P, HC]
    vmask = np.zeros((P, HC, QSTRIP, QSTRIP), dtype=np.float16)
    di = np.arange(QSTRIP)
    vmask[:, :, di, di] = vc[:, :, None]

    in_maps = []
    idxs = []
    for b in range(B):
        idx = np.nonzero(~mask[b])[0]
        mu_b = len(idx)
        idx_pad = np.concatenate([idx, np.zeros(MU - mu_b, dtype=idx.dtype)])
        mem_c = np.ascontiguousarray(memory[b][idx_pad])  # [MU, M_SIZE]
        maskval = np.zeros(MU, dtype=np.float32)
        maskval[mu_b:] = MASKED_VALUE  # pad columns excluded from softmax
        maskbc = np.ascontiguousarray(np.broadcast_to(maskval, (LQ, MU)))
        in_maps.append(
            {
                "query": query[b],
                "memory": mem_c,
                "maskval": maskbc,
                "Wq": Wq,
                "Wm": Wm,
                "bqc": bqc,
                "vmask": vmask,
            }
        )
        idxs.append((idx, mu_b))
    return in_maps, idxs


def _run(inputs, trace=False):
    """Run on 8 NeuronCores; returns ((weighted_memory, weights), exec_time_ns)."""
    from concourse.bass_utils import run_bass_kernel_spmd

    mask = np.asarray(inputs["mask"]).astype(bool)
    MU = _choose_mu(mask)
    nc = _get_nc(MU)
    in_maps, idxs = _prep_in_maps(**inputs, MU=MU)
    res = run_bass_kernel_spmd(nc, in_maps, core_ids=list(range(B)), trace=trace)
    wm = np.stack([r["wm_out"] for r in res.results]).astype(np.float32)
    w = np.zeros((B, LQ, LM), dtype=np.float32)
    for b in range(B):
        idx, mu_b = idxs[b]
        w[b][:, idx] = res.results[b]["w_out"][:, :mu_b]
    return (wm, w), res.exec_time_ns


def kernel(query, memory, mask, Wq, bq, Wm, v):
    (wm, w), _ = _run(
        dict(query=query, memory=memory, mask=mask, Wq=Wq, bq=bq, Wm=Wm, v=v),
        trace=bool(int(os.environ.get("KERNEL_TRACE", "0"))),
    )
    return wm, w


if __name__ == "__main__":
    nc = _get_nc(384)
    print("built ok:", nc.name)


# revision 33
# speedup vs baseline: 1.5208x; 1.0546x over previous
"""Bahdanau (MLP) attention kernel for Trainium2, data-parallel over batch.

reference math (per batch b):
    q_proj = query @ Wq + bq                     [Lq, H]
    k_proj = memory @ Wm                         [Lm, H]
    attn[q, m] = sum_h v[h] * tanh(q_proj[q, h] + k_proj[m, h])
    attn = where(mask[m], -1e24, attn)
    weights = softmax(attn, axis=-1)             [Lq, Lm]
    weighted_memory = weights @ memory           [Lq, Ms]
    returns (weighted_memory, weights)

Shapes hardcoded: B=8, Lq=128, Lm=512, Q=M=512, H=256, fp32. One batch per
NeuronCore (8 cores, SPMD).

Masked memory positions receive softmax weight exactly 0 (exp(-1e24) == 0 in
fp32), so their tanh columns never affect either output. The host gathers the
unmasked memory rows (a mask-derived layout transform), the device computes
attention over MU = ceil(max_unmasked/128)*128 compacted columns, and the host
scatters the compact weights back to [Lq, Lm] (masked entries = 0 exactly, as
in the reference). weighted_memory comes out of the device already full-width.

Device pipeline (per core):
  - k_projT [h, mu] and q_projT [h, q] with h on partitions (2 chunks of 128),
    via PE transposes + fp32r matmuls (query/memory/Wq/Wm declared float32r).
  - main loop over groups of GQ q's: DVE pre-adds z = k_projT + q_projT[:, q]
    (fp16 out, 4x mode), one big ACT tanh per group (fp16), PE accumulates
    attn[q, mu] = sum_h v_h * tanh into one PSUM bank using [128, 32] fp16
    "masked v" stationaries (v in column q%32) so each matmul writes one
    32-partition strip; zero columns accumulate exactly.
  - epilogue: +pad-mask, softmax without max-subtraction (|attn| < sum|v| < 16
    so exp cannot overflow; -1e24 still underflows to 0), PE transpose of
    weights, fp32r matmul weights @ memory_compact -> full [Lq, Ms].
"""

import functools
import os

import numpy as np

B, LQ, LM = 8, 128, 512
Q_SIZE, M_SIZE, H_SIZE = 512, 512, 256
MASKED_VALUE = -1e24
P = 128
HC = H_SIZE // P  # 2 h-chunks
DC = Q_SIZE // P  # 4 d-chunks
GQ = 8            # q's per tanh batch
NG = LQ // GQ
QSTRIP = 32       # PE col-tiling strip


def _build_nc(MU):
    import concourse.mybir as mybir
    import concourse.tile as tile
    from concourse import bacc
    from concourse.masks import make_identity

    f32 = mybir.dt.float32
    f32r = mybir.dt.float32r
    f16 = mybir.dt.float16
    AF = mybir.ActivationFunctionType
    AX = mybir.AxisListType

    MUC = MU // P  # compacted m-chunks

    nc = bacc.Bacc("TRN2", name="mlp_attn")

    q_d = nc.dram_tensor("query", [LQ, Q_SIZE], f32r, kind="ExternalInput")
    m_d = nc.dram_tensor("memory", [MU, M_SIZE], f32r, kind="ExternalInput")
    mask_d = nc.dram_tensor("maskval", [LQ, MU], f32, kind="ExternalInput")
    wq_d = nc.dram_tensor("Wq", [Q_SIZE, H_SIZE], f32r, kind="ExternalInput")
    wm_d = nc.dram_tensor("Wm", [M_SIZE, H_SIZE], f32r, kind="ExternalInput")
    bq_d = nc.dram_tensor("bqc", [P, HC], f32, kind="ExternalInput")
    vmask_d = nc.dram_tensor("vmask", [P, HC, QSTRIP, QSTRIP], f16, kind="ExternalInput")
    wmo_d = nc.dram_tensor("wm_out", [LQ, M_SIZE], f32, kind="ExternalOutput")
    wo_d = nc.dram_tensor("w_out", [LQ, MU], f32, kind="ExternalOutput")

    with tile.TileContext(nc) as tc:
        with (
            tc.tile_pool(name="const", bufs=1) as cpool,
            tc.tile_pool(name="io", bufs=1) as iopool,
            tc.tile_pool(name="work", bufs=1) as wpool,
            tc.tile_pool(name="z", bufs=3) as zpool,
            tc.tile_pool(name="th", bufs=3) as thpool,
            tc.tile_pool(name="ps", bufs=2, space="PSUM") as pspool,
            tc.tile_pool(name="tp", bufs=3, space="PSUM") as tppool,
            tc.tile_pool(name="attnps", bufs=1, space="PSUM") as apool,
            tc.tile_pool(name="outps", bufs=1, space="PSUM") as opool,
        ):
            # ---------------- constants ----------------
            ident = cpool.tile([P, P], f32)
            make_identity(nc, ident[:])
            ident_r = cpool.tile([P, P], f32r)
            nc.vector.tensor_copy(ident_r[:], ident[:])

            # preload the exp_and_others ACT table set (tanh + exp) at t=0
            warm = cpool.tile([P, 1], f32)
            nc.vector.memset(warm[:], 0.0)
            nc.scalar.activation(warm[:], warm[:], AF.Tanh)

            ones_row = cpool.tile([1, P], f32)
            nc.vector.memset(ones_row[:], 1.0)

            # DMA order matches PE consumption order: memory/Wm gate the long
            # k-projection chain, so they go first; query/Wq follow
            mem_sb = iopool.tile([P, MUC, M_SIZE], f32r)
            for mc in range(MUC):
                nc.sync.dma_start(mem_sb[:, mc, :], m_d[mc * P : (mc + 1) * P, :])
            wm_sb = iopool.tile([P, DC, H_SIZE], f32r)
            nc.sync.dma_start(wm_sb[:], wm_d.rearrange("(dc p) h -> p dc h", p=P))
            qry_sb = iopool.tile([P, Q_SIZE], f32r)
            nc.sync.dma_start(qry_sb[:], q_d[:])
            wq_sb = iopool.tile([P, DC, H_SIZE], f32r)
            nc.sync.dma_start(wq_sb[:], wq_d.rearrange("(dc p) h -> p dc h", p=P))
            bq_sb = cpool.tile([P, HC], f32)
            nc.sync.dma_start(bq_sb[:], bq_d[:])

            # masked-v stationaries: vmask[hc][r][:, c] = v_chunk_hc if c == r else 0
            vmask = cpool.tile([P, HC, QSTRIP, QSTRIP], f16)
            nc.sync.dma_start(vmask[:], vmask_d[:])
            maskbc = iopool.tile([P, MU], f32)
            nc.sync.dma_start(maskbc[:], mask_d[:])

            # ---------------- memoryT [d, mu] + kpT [h, mu] (f16) ----------
            memT = wpool.tile([P, DC, MU], f32r)
            for dc in range(DC):
                for mc in range(MUC):
                    tp = tppool.tile([P, P], f32r, tag="tp")
                    nc.tensor.transpose(
                        tp[:], mem_sb[:, mc, dc * P : (dc + 1) * P], ident_r[:]
                    )
                    nc.vector.tensor_copy(memT[:, dc, mc * P : (mc + 1) * P], tp[:])
            kpT = wpool.tile([P, HC, MU], f16)
            for hc in range(HC):
                pt = pspool.tile([P, M_SIZE], f32, tag="proj_psum")
                for dc in range(DC):
                    nc.tensor.matmul(
                        pt[:, :MU],
                        wm_sb[:, dc, hc * P : (hc + 1) * P],
                        memT[:, dc, :],
                        start=(dc == 0),
                        stop=(dc == DC - 1),
                    )
                nc.vector.tensor_copy(kpT[:, hc, :], pt[:, :MU])

            # ---------------- queryT [d, q]; qpT [h, q] (+bq) ----------------
            qryT = wpool.tile([P, DC, LQ], f32r)
            for dc in range(DC):
                tp = tppool.tile([P, P], f32r, tag="tp")
                nc.tensor.transpose(tp[:], qry_sb[:, dc * P : (dc + 1) * P], ident_r[:])
                nc.vector.tensor_copy(qryT[:, dc, :], tp[:])
            qpT = wpool.tile([P, HC, LQ], f32)
            for hc in range(HC):
                pt = pspool.tile([P, M_SIZE], f32, tag="proj_psum")
                for dc in range(DC):
                    nc.tensor.matmul(
                        pt[:, :LQ],
                        wq_sb[:, dc, hc * P : (hc + 1) * P],
                        qryT[:, dc, :],
                        start=(dc == 0),
                        stop=(dc == DC - 1),
                    )
                nc.vector.tensor_scalar_add(
                    qpT[:, hc, :], pt[:, :LQ], bq_sb[:, hc : hc + 1]
                )

            # ---------------- main loop: attn[q, mu] in one PSUM bank ----------------
            # taper the final groups so the last tanh -> last matmul -> softmax
            # chain in the epilogue is short
            group_sizes = (
                [1, 1, 2, 4] + [GQ] * (LQ // GQ - 2) + [GQ // 2, GQ // 4, 1, 1]
            )
            assert sum(group_sizes) == LQ
            attn_ps = apool.tile([P, MU], f32)
            q0 = 0
            for gq in group_sizes:
                z = zpool.tile([P, GQ * HC, MU], f16, tag="z")
                for ql in range(gq):
                    q = q0 + ql
                    for hc in range(HC):
                        nc.vector.tensor_scalar_add(
                            z[:, ql * HC + hc, :],
                            kpT[:, hc, :],
                            qpT[:, hc, q : q + 1],
                        )
                th = thpool.tile([P, GQ * HC, MU], f16, tag="th")
                if gq == 1:
                    # split per h-chunk so the first tanh only waits for hc0
                    for hc in range(HC):
                        nc.scalar.activation(
                            th[:, hc : hc + 1, :], z[:, hc : hc + 1, :], AF.Tanh
                        )
                else:
                    nc.scalar.activation(
                        th[:, : gq * HC, :], z[:, : gq * HC, :], AF.Tanh
                    )
                for ql in range(gq):
                    q = q0 + ql
                    s, r = divmod(q, QSTRIP)
                    for hc in range(HC):
                        first = (q % QSTRIP == 0) and hc == 0
                        last = (q % QSTRIP == QSTRIP - 1) and hc == HC - 1
                        nc.tensor.matmul(
                            attn_ps[s * QSTRIP : (s + 1) * QSTRIP, :],
                            vmask[:, hc, r, :],
                            th[:, ql * HC + hc, :],
                            start=first,
                            stop=last,
                            tile_position=(0, s * QSTRIP),
                        )
                        if first:
                            # fold the pad-mask into the PSUM accumulation
                            nc.tensor.matmul(
                                attn_ps[s * QSTRIP : (s + 1) * QSTRIP, :],
                                ones_row[:, s * QSTRIP : (s + 1) * QSTRIP],
                                maskbc[0:1, :],
                                start=False,
                                stop=False,
                                tile_position=(0, s * QSTRIP),
                            )
                q0 += gq

            # ---------------- softmax (no max-subtraction; |attn| < 16) ----------
            esb = wpool.tile([P, MU], f32)
            nc.scalar.activation(esb[:], attn_ps[:], AF.Exp)
            sm = wpool.tile([P, 1], f32)
            nc.vector.reduce_sum(sm[:], esb[:], axis=AX.X)
            rs = wpool.tile([P, 1], f32)
            nc.vector.reciprocal(rs[:], sm[:])

            # weighted_memory = (exp @ memory_compact) * (1/rowsum): the per-row
            # normalization commutes with the matmul, so the exp transposes can
            # start without waiting for sum/recip
            eT = wpool.tile([P, MUC, LQ], f32r)
            for mc in range(MUC):
                tp = tppool.tile([P, P], f32, tag="tp")
                nc.tensor.transpose(tp[:], esb[:, mc * P : (mc + 1) * P], ident[:])
                nc.vector.tensor_copy(eT[:, mc, :], tp[:])
            out_ps = opool.tile([P, M_SIZE], f32)
            for mc in range(MUC):
                nc.tensor.matmul(
                    out_ps[:],
                    eT[:, mc, :],
                    mem_sb[:, mc, :],
                    start=(mc == 0),
                    stop=(mc == MUC - 1),
                )
            out_sb = wpool.tile([P, M_SIZE], f32)
            nc.vector.tensor_scalar_mul(out_sb[:], out_ps[:], rs[:])
            nc.sync.dma_start(wmo_d[:], out_sb[:])

            # normalized weights output (off the critical chain)
            w_sb = wpool.tile([P, MU], f32)
            nc.vector.tensor_scalar_mul(w_sb[:], esb[:], rs[:])
            nc.sync.dma_start(wo_d[:], w_sb[:])

    nc.compile()
    return nc


@functools.lru_cache(maxsize=2)
def _get_nc(MU=LM):
    return _build_nc(MU)


def _choose_mu(mask):
    """Smallest multiple of 128 covering every batch's unmasked count."""
    mu_max = int((~mask).sum(axis=-1).max())
    mu = max(P, -(-mu_max // P) * P)
    return min(mu, LM)


def _prep_in_maps(query, memory, mask, Wq, bq, Wm, v, MU):
    query = np.ascontiguousarray(np.asarray(query, dtype=np.float32))
    memory = np.ascontiguousarray(np.asarray(memory, dtype=np.float32))
    mask = np.asarray(mask).astype(bool)
    Wq = np.ascontiguousarray(np.asarray(Wq, dtype=np.float32))
    Wm = np.ascontiguousarray(np.asarray(Wm, dtype=np.float32))
    bq = np.asarray(bq, dtype=np.float32)
    v = np.asarray(v, dtype=np.float32)

    bqc = np.ascontiguousarray(bq.reshape(HC, P).T)  # [P, HC]
    vc = np.ascontiguousarray(v.reshape(HC, P).T)  # [P, HC]
    vmask = np.zeros((P, HC, QSTRIP, QSTRIP), dtype=np.float16)
    di = np.arange(QSTRIP)
    vmask[:, :, di, di] = vc[:, :, None]

    in_maps = []
    idxs = []
    for b in range(B):
        idx = np.nonzero(~mask[b])[0]
        mu_b = len(idx)
        idx_pad = np.concatenate([idx, np.zeros(MU - mu_b, dtype=idx.dtype)])
        mem_c = np.ascontiguousarray(memory[b][idx_pad])  # [MU, M_SIZE]
        maskval = np.zeros(MU, dtype=np.float32)
        maskval[mu_b:] = MASKED_VALUE  # pad columns excluded from softmax
        maskbc = np.ascontiguousarray(np.broadcast_to(maskval, (LQ, MU)))
        in_maps.append(
            {
                "query": query[b],
                "memory": mem_c,
                "maskval": maskbc,
                "Wq": Wq,
                "Wm": Wm,
                "bqc": bqc,
                "vmask": vmask,
            }
        )
        idxs.append((idx, mu_b))
    return in_maps, idxs


def _run(inputs, trace=False):
    """Run on 8 NeuronCores; returns ((weighted_memory, weights), exec_time_ns)."""
    from concourse.bass_utils import run_bass_kernel_spmd

    mask = np.asarray(inputs["mask"]).astype(bool)
    MU = _choose_mu(mask)
    nc = _get_nc(MU)
    in_maps, idxs = _prep_in_maps(**inputs, MU=MU)
    res = run_bass_kernel_spmd(nc, in_maps, core_ids=list(range(B)), trace=trace)
    wm = np.stack([r["wm_out"] for r in res.results]).astype(np.float32)
    w = np.zeros((B, LQ, LM), dtype=np.float32)
    for b in range(B):
        idx, mu_b = idxs[b]
        w[b][:, idx] = res.results[b]["w_out"][:, :mu_b]
    return (wm, w), res.exec_time_ns


def kernel(query, memory, mask, Wq, bq, Wm, v):
    (wm, w), _ = _run(
        dict(query=query, memory=memory, mask=mask, Wq=Wq, bq=bq, Wm=Wm, v=v),
        trace=bool(int(os.environ.get("KERNEL_TRACE", "0"))),
    )
    return wm, w


if __name__ == "__main__":
    nc = _get_nc(384)
    print("built ok:", nc.name)


# revision 36
# speedup vs baseline: 1.5309x; 1.0067x over previous
"""Bahdanau (MLP) attention kernel for Trainium2, data-parallel over batch.

reference math (per batch b):
    q_proj = query @ Wq + bq                     [Lq, H]
    k_proj = memory @ Wm                         [Lm, H]
    attn[q, m] = sum_h v[h] * tanh(q_proj[q, h] + k_proj[m, h])
    attn = where(mask[m], -1e24, attn)
    weights = softmax(attn, axis=-1)             [Lq, Lm]
    weighted_memory = weights @ memory           [Lq, Ms]
    returns (weighted_memory, weights)

Shapes hardcoded: B=8, Lq=128, Lm=512, Q=M=512, H=256, fp32. One batch per
NeuronCore (8 cores, SPMD).

Masked memory positions receive softmax weight exactly 0 (exp(-1e24) == 0 in
fp32), so their tanh columns never affect either output. The host gathers the
unmasked memory rows (a mask-derived layout transform), the device computes
attention over MU = ceil(max_unmasked/128)*128 compacted columns, and the host
scatters the compact weights back to [Lq, Lm] (masked entries = 0 exactly, as
in the reference). weighted_memory comes out of the device already full-width.

Device pipeline (per core):
  - k_projT [h, mu] and q_projT [h, q] with h on partitions (2 chunks of 128),
    via PE transposes + fp32r matmuls (query/memory/Wq/Wm declared float32r).
  - main loop over groups of GQ q's: DVE pre-adds z = k_projT + q_projT[:, q]
    (fp16 out, 4x mode), one big ACT tanh per group (fp16), PE accumulates
    attn[q, mu] = sum_h v_h * tanh into one PSUM bank using [128, 32] fp16
    "masked v" stationaries (v in column q%32) so each matmul writes one
    32-partition strip; zero columns accumulate exactly.
  - epilogue: +pad-mask, softmax without max-subtraction (|attn| < sum|v| < 16
    so exp cannot overflow; -1e24 still underflows to 0), PE transpose of
    weights, fp32r matmul weights @ memory_compact -> full [Lq, Ms].
"""

import functools
import os

import numpy as np

B, LQ, LM = 8, 128, 512
Q_SIZE, M_SIZE, H_SIZE = 512, 512, 256
MASKED_VALUE = -1e24
P = 128
HC = H_SIZE // P  # 2 h-chunks
DC = Q_SIZE // P  # 4 d-chunks
GQ = 8            # q's per tanh batch
NG = LQ // GQ
QSTRIP = 32       # PE col-tiling strip


def _build_nc(MU):
    import concourse.mybir as mybir
    import concourse.tile as tile
    from concourse import bacc
    from concourse.masks import make_identity

    f32 = mybir.dt.float32
    f32r = mybir.dt.float32r
    f16 = mybir.dt.float16
    AF = mybir.ActivationFunctionType
    AX = mybir.AxisListType

    MUC = MU // P  # compacted m-chunks

    nc = bacc.Bacc("TRN2", name="mlp_attn")

    q_d = nc.dram_tensor("query", [LQ, Q_SIZE], f32r, kind="ExternalInput")
    m_d = nc.dram_tensor("memory", [MU, M_SIZE], f32r, kind="ExternalInput")
    mask_d = nc.dram_tensor("maskval", [LQ, MU], f32, kind="ExternalInput")
    wq_d = nc.dram_tensor("Wq", [Q_SIZE, H_SIZE], f32r, kind="ExternalInput")
    wm_d = nc.dram_tensor("Wm", [M_SIZE, H_SIZE], f32r, kind="ExternalInput")
    bq_d = nc.dram_tensor("bqc", [P, HC], f32, kind="ExternalInput")
    vmask_d = nc.dram_tensor("vmask", [P, HC, QSTRIP, QSTRIP], f16, kind="ExternalInput")
    wmo_d = nc.dram_tensor("wm_out", [LQ, M_SIZE], f32, kind="ExternalOutput")
    wo_d = nc.dram_tensor("w_out", [LQ, MU], f32, kind="ExternalOutput")

    with tile.TileContext(nc) as tc:
        with (
            tc.tile_pool(name="const", bufs=1) as cpool,
            tc.tile_pool(name="io", bufs=1) as iopool,
            tc.tile_pool(name="work", bufs=1) as wpool,
            tc.tile_pool(name="z", bufs=3) as zpool,
            tc.tile_pool(name="th", bufs=3) as thpool,
            tc.tile_pool(name="ps", bufs=2, space="PSUM") as pspool,
            tc.tile_pool(name="tp", bufs=3, space="PSUM") as tppool,
            tc.tile_pool(name="attnps", bufs=1, space="PSUM") as apool,
            tc.tile_pool(name="outps", bufs=1, space="PSUM") as opool,
        ):
            # ---------------- constants ----------------
            ident = cpool.tile([P, P], f32)
            make_identity(nc, ident[:])
            ident_r = cpool.tile([P, P], f32r)
            nc.vector.tensor_copy(ident_r[:], ident[:])

            # preload the exp_and_others ACT table set (tanh + exp) at t=0
            warm = cpool.tile([P, 1], f32)
            nc.vector.memset(warm[:], 0.0)
            nc.scalar.activation(warm[:], warm[:], AF.Tanh)

            ones_row = cpool.tile([1, P], f32)
            nc.vector.memset(ones_row[:], 1.0)

            # PE warmup: dummy identity transposes bridge the DMA wait so the
            # PE clock ramp (3us of continuous busy -> full speed) is already
            # done when the real prologue matmuls arrive
            for _ in range(10):
                warm_ps = tppool.tile([P, P], f32, tag="tp")
                nc.tensor.matmul(warm_ps[:], ident_r[:], ident_r[:])

            # DMA order matches PE consumption order: memory/Wm gate the long
            # k-projection chain, so they go first; query/Wq follow
            mem_sb = iopool.tile([P, MUC, M_SIZE], f32r)
            for mc in range(MUC):
                nc.sync.dma_start(mem_sb[:, mc, :], m_d[mc * P : (mc + 1) * P, :])
            wm_sb = iopool.tile([P, DC, H_SIZE], f32r)
            nc.sync.dma_start(wm_sb[:], wm_d.rearrange("(dc p) h -> p dc h", p=P))
            qry_sb = iopool.tile([P, Q_SIZE], f32r)
            nc.sync.dma_start(qry_sb[:], q_d[:])
            wq_sb = iopool.tile([P, DC, H_SIZE], f32r)
            nc.sync.dma_start(wq_sb[:], wq_d.rearrange("(dc p) h -> p dc h", p=P))
            bq_sb = cpool.tile([P, HC], f32)
            nc.sync.dma_start(bq_sb[:], bq_d[:])

            # masked-v stationaries: vmask[hc][r][:, c] = v_chunk_hc if c == r else 0
            vmask = cpool.tile([P, HC, QSTRIP, QSTRIP], f16)
            nc.sync.dma_start(vmask[:], vmask_d[:])
            maskbc = iopool.tile([P, MU], f32)
            nc.sync.dma_start(maskbc[:], mask_d[:])

            # ---------------- memoryT [d, mu] + kpT [h, mu] (f16) ----------
            memT = wpool.tile([P, DC, MU], f32r)
            for dc in range(DC):
                for mc in range(MUC):
                    tp = tppool.tile([P, P], f32r, tag="tp")
                    nc.tensor.transpose(
                        tp[:], mem_sb[:, mc, dc * P : (dc + 1) * P], ident_r[:]
                    )
                    nc.vector.tensor_copy(memT[:, dc, mc * P : (mc + 1) * P], tp[:])
            kpT = wpool.tile([P, HC, MU], f16)
            for hc in range(HC):
                pt = pspool.tile([P, M_SIZE], f32, tag="proj_psum")
                for dc in range(DC):
                    nc.tensor.matmul(
                        pt[:, :MU],
                        wm_sb[:, dc, hc * P : (hc + 1) * P],
                        memT[:, dc, :],
                        start=(dc == 0),
                        stop=(dc == DC - 1),
                    )
                nc.vector.tensor_copy(kpT[:, hc, :], pt[:, :MU])

            # ---------------- queryT [d, q]; qpT [h, q] (+bq) ----------------
            qryT = wpool.tile([P, DC, LQ], f32r)
            for dc in range(DC):
                tp = tppool.tile([P, P], f32r, tag="tp")
                nc.tensor.transpose(tp[:], qry_sb[:, dc * P : (dc + 1) * P], ident_r[:])
                nc.vector.tensor_copy(qryT[:, dc, :], tp[:])
            qpT = wpool.tile([P, HC, LQ], f32)
            for hc in range(HC):
                pt = pspool.tile([P, M_SIZE], f32, tag="proj_psum")
                for dc in range(DC):
                    nc.tensor.matmul(
                        pt[:, :LQ],
                        wq_sb[:, dc, hc * P : (hc + 1) * P],
                        qryT[:, dc, :],
                        start=(dc == 0),
                        stop=(dc == DC - 1),
                    )
                nc.vector.tensor_scalar_add(
                    qpT[:, hc, :], pt[:, :LQ], bq_sb[:, hc : hc + 1]
                )

            # ---------------- main loop: attn[q, mu] in one PSUM bank ----------------
            # taper the final groups so the last tanh -> last matmul -> softmax
            # chain in the epilogue is short
            group_sizes = (
                [1, 1, 2, 4] + [GQ] * (LQ // GQ - 2) + [GQ // 2, GQ // 4, 1, 1]
            )
            assert sum(group_sizes) == LQ
            attn_ps = apool.tile([P, MU], f32)
            q0 = 0
            for gq in group_sizes:
                z = zpool.tile([P, GQ * HC, MU], f16, tag="z")
                for ql in range(gq):
                    q = q0 + ql
                    for hc in range(HC):
                        nc.vector.tensor_scalar_add(
                            z[:, ql * HC + hc, :],
                            kpT[:, hc, :],
                            qpT[:, hc, q : q + 1],
                        )
                th = thpool.tile([P, GQ * HC, MU], f16, tag="th")
                if gq == 1:
                    # split per h-chunk so the first tanh only waits for hc0
                    for hc in range(HC):
                        nc.scalar.activation(
                            th[:, hc : hc + 1, :], z[:, hc : hc + 1, :], AF.Tanh
                        )
                else:
                    nc.scalar.activation(
                        th[:, : gq * HC, :], z[:, : gq * HC, :], AF.Tanh
                    )
                for ql in range(gq):
                    q = q0 + ql
                    s, r = divmod(q, QSTRIP)
                    for hc in range(HC):
                        first = (q % QSTRIP == 0) and hc == 0
                        last = (q % QSTRIP == QSTRIP - 1) and hc == HC - 1
                        nc.tensor.matmul(
                            attn_ps[s * QSTRIP : (s + 1) * QSTRIP, :],
                            vmask[:, hc, r, :],
                            th[:, ql * HC + hc, :],
                            start=first,
                            stop=last,
                            tile_position=(0, s * QSTRIP),
                        )
                        if first:
                            # fold the pad-mask into the PSUM accumulation
                            nc.tensor.matmul(
                                attn_ps[s * QSTRIP : (s + 1) * QSTRIP, :],
                                ones_row[:, s * QSTRIP : (s + 1) * QSTRIP],
                                maskbc[0:1, :],
                                start=False,
                                stop=False,
                                tile_position=(0, s * QSTRIP),
                            )
                q0 += gq

            # ---------------- softmax (no max-subtraction; |attn| < 16) ----------
            esb = wpool.tile([P, MU], f32)
            nc.scalar.activation(esb[:], attn_ps[:], AF.Exp)
            sm = wpool.tile([P, 1], f32)
            nc.vector.reduce_sum(sm[:], esb[:], axis=AX.X)
            rs = wpool.tile([P, 1], f32)
            nc.vector.reciprocal(rs[:], sm[:])

            # weighted_memory = (exp @ memory_compact) * (1/rowsum): the per-row
            # normalization commutes with the matmul, so the exp transposes can
            # start without waiting for sum/recip
            eT = wpool.tile([P, MUC, LQ], f32r)
            for mc in range(MUC):
                tp = tppool.tile([P, P], f32, tag="tp")
                nc.tensor.transpose(tp[:], esb[:, mc * P : (mc + 1) * P], ident[:])
                nc.vector.tensor_copy(eT[:, mc, :], tp[:])
            out_ps = opool.tile([P, M_SIZE], f32)
            for mc in range(MUC):
                nc.tensor.matmul(
                    out_ps[:],
                    eT[:, mc, :],
                    mem_sb[:, mc, :],
                    start=(mc == 0),
                    stop=(mc == MUC - 1),
                )
            out_sb = wpool.tile([P, M_SIZE], f32)
            nc.vector.tensor_scalar_mul(out_sb[:], out_ps[:], rs[:])
            nc.sync.dma_start(wmo_d[:], out_sb[:])

            # normalized weights output (off the critical chain)
            w_sb = wpool.tile([P, MU], f32)
            nc.vector.tensor_scalar_mul(w_sb[:], esb[:], rs[:])
            nc.sync.dma_start(wo_d[:], w_sb[:])

    nc.compile()
    return nc


@functools.lru_cache(maxsize=2)
def _get_nc(MU=LM):
    return _build_nc(MU)


def _choose_mu(mask):
    """Smallest multiple of 128 covering every batch's unmasked count."""
    mu_max = int((~mask).sum(axis=-1).max())
    mu = max(P, -(-mu_max // P) * P)
    return min(mu, LM)


def _prep_in_maps(query, memory, mask, Wq, bq, Wm, v, MU):
    query = np.ascontiguousarray(np.asarray(query, dtype=np.float32))
    memory = np.ascontiguousarray(np.asarray(memory, dtype=np.float32))
    mask = np.asarray(mask).astype(bool)
    Wq = np.ascontiguousarray(np.asarray(Wq, dtype=np.float32))
    Wm = np.ascontiguousarray(np.asarray(Wm, dtype=np.float32))
    bq = np.asarray(bq, dtype=np.float32)
    v = np.asarray(v, dtype=np.float32)

    bqc = np.ascontiguousarray(bq.reshape(HC, P).T)  # [P, HC]
    vc = np.ascontiguousarray(v.reshape(HC, P).T)  # [P, HC]
    vmask = np.zeros((P, HC, QSTRIP, QSTRIP), dtype=np.float16)
    di = np.arange(QSTRIP)
    vmask[:, :, di, di] = vc[:, :, None]

    in_maps = []
    idxs = []
    for b in range(B):
        idx = np.nonzero(~mask[b])[0]
        mu_b = len(idx)
        idx_pad = np.concatenate([idx, np.zeros(MU - mu_b, dtype=idx.dtype)])
        mem_c = np.ascontiguousarray(memory[b][idx_pad])  # [MU, M_SIZE]
        maskval = np.zeros(MU, dtype=np.float32)
        maskval[mu_b:] = MASKED_VALUE  # pad columns excluded from softmax
        maskbc = np.ascontiguousarray(np.broadcast_to(maskval, (LQ, MU)))
        in_maps.append(
            {
                "query": query[b],
                "memory": mem_c,
                "maskval": maskbc,
                "Wq": Wq,
                "Wm": Wm,
                "bqc": bqc,
                "vmask": vmask,
            }
        )
        idxs.append((idx, mu_b))
    return in_maps, idxs


def _run(inputs, trace=False):
    """Run on 8 NeuronCores; returns ((weighted_memory, weights), exec_time_ns)."""
    from concourse.bass_utils import run_bass_kernel_spmd

    mask = np.asarray(inputs["mask"]).astype(bool)
    MU = _choose_mu(mask)
    nc = _get_nc(MU)
    in_maps, idxs = _prep_in_maps(**inputs, MU=MU)
    res = run_bass_kernel_spmd(nc, in_maps, core_ids=list(range(B)), trace=trace)
    wm = np.stack([r["wm_out"] for r in res.results]).astype(np.float32)
    w = np.zeros((B, LQ, LM), dtype=np.float32)
    for b in range(B):
        idx, mu_b = idxs[b]
        w[b][:, idx] = res.results[b]["w_out"][:, :mu_b]
    return (wm, w), res.exec_time_ns


def kernel(query, memory, mask, Wq, bq, Wm, v):
    (wm, w), _ = _run(
        dict(query=query, memory=memory, mask=mask, Wq=Wq, bq=bq, Wm=Wm, v=v),
        trace=bool(int(os.environ.get("KERNEL_TRACE", "0"))),
    )
    return wm, w


if __name__ == "__main__":
    nc = _get_nc(384)
    print("built ok:", nc.name)
